# revision 22
# baseline (speedup 1.0000x reference)
"""ContextualRoIAlign Trainium2 kernel (v2: bf16 + streamed stage-2).

Problem (hardcoded): B=2, C=256, H=W=56, N=64 boxes, M=8 gt boxes, P=7.
out[b,n,c,p,q] = roi_align(fm[b], box_n)[c,p,q]
                 + mean_m roi_align(fm[b], union(box_n, gt_m))[c,p,q]

Decomposition: roi_align separates per axis into small interpolation
matrices Ay, Ax ([R,7,56], host-precomputed exactly like the reference):
  out[r,c,p,q] = sum_h Ay[r,p,h] * (sum_w fm[c,h,w] * Ax[r,q,w])
The 1/M mean weight is folded into Ax of the context rois, and the 9-roi
group sum is accumulated in PSUM.

Sharding: 8 cores; core k handles image k//4, box groups [16*(k%4), +16)
=> 144 rois per core (16 groups x (1 box + 8 ctx)). fm replicated per
image (4 cores each).

Device program per core (bf16 matmul inputs, fp32 psum accumulate):
  Stage 1 (contract h): weights = fm channel-pair [h=56, 128] where col
    c_loc*64+w holds fm[2i+c_loc, h, w]; rhs = AyT [56, 504] (72 rois'
    (r,p) columns per chunk); psum [128, 504] -> TMP[128, 72, 112] bf16.
    TMP partition c_loc*64+w holds tmp[c=2*(win*16+il)+c_loc, w] at free
    col (r_loc, il*7+p): w on partitions, no inter-stage transpose.
  Stage 2 (contract w): stationary = AxBD[:, r*16:+16] ([128,16] block-
    diagonal: rows 0:56 have Ax[r] in cols 0:8, rows 64:120 have Ax[r]
    in cols 8:16 => the 16-col LDWEIGHTS hides under the previous
    matmul); moving = TMP[:, r_loc, :] ([128, 112]); out [16,112].
    9 rois of a group accumulate in one psum chain; 4 groups share one
    psum tile at PE col-tile offsets 0/32/64/96.
"""
import os
import numpy as np
import ml_dtypes

P = 7
B, C, H, W, N, M = 2, 256, 56, 56, 64, 8
NCORES = 8
GROUPS_PER_CORE = 16
ROIS_PER_GROUP = 9
R_CORE = GROUPS_PER_CORE * ROIS_PER_GROUP   # 144
RP = R_CORE * P                              # 1008
WIN = 16                                     # channel pairs per window
NWIN = 128 // WIN                            # 8
NCHUNK = 504                                 # stage-1 rhs cols per matmul
RCHUNK = NCHUNK // P                         # 72 rois per TMP tile

BF16 = ml_dtypes.bfloat16


# ---------------------------------------------------------------- host prep

def _axis_weights(start, length, dim):
    """Exact numpy port of the reference's _axis_weights (float32)."""
    start = start.astype(np.float32)
    length = length.astype(np.float32)
    R = start.shape[0]
    S = int(np.ceil(dim / P))
    bin_sz = length / np.float32(P)
    grid = np.ceil(length / np.float32(P)).astype(np.int32)
    g = grid.astype(np.float32)[:, None, None]
    s = np.arange(S, dtype=np.float32)
    ph = np.arange(P, dtype=np.float32)
    coord = (start[:, None, None] + ph[None, :, None] * bin_sz[:, None, None]
             + (s[None, None, :] + np.float32(0.5)) * bin_sz[:, None, None] / g)
    valid = (coord >= -1.0) & (coord <= dim)
    c = np.maximum(coord, np.float32(0.0))
    low = np.floor(c).astype(np.int32)
    hi_clamp = low >= dim - 1
    low = np.where(hi_clamp, dim - 1, low)
    high = np.where(hi_clamp, dim - 1, low + 1)
    cv = np.where(hi_clamp, low.astype(np.float32), c)
    l = cv - low.astype(np.float32)
    smask = (s[None, None, :] < g) & valid
    w = smask.astype(np.float32) / g
    w_low = ((np.float32(1.0) - l) * w).astype(np.float32)
    w_high = (l * w).astype(np.float32)
    A = np.zeros((R, P, dim), dtype=np.float32)
    r_idx = np.broadcast_to(np.arange(R)[:, None, None], low.shape)
    p_idx = np.broadcast_to(np.arange(P)[None, :, None], low.shape)
    np.add.at(A, (r_idx, p_idx, low), w_low)
    np.add.at(A, (r_idx, p_idx, high), w_high)
    return A


def _prep_core(fm_b, boxes_b, gt_b, g0):
    b = boxes_b.astype(np.float32)
    g = gt_b.astype(np.float32)
    x1 = np.minimum(b[:, None, 0], g[None, :, 0])
    y1 = np.minimum(b[:, None, 1], g[None, :, 1])
    x2 = np.maximum(b[:, None, 2], g[None, :, 2])
    y2 = np.maximum(b[:, None, 3], g[None, :, 3])
    ctx = np.stack([x1, y1, x2, y2], axis=-1)                 # [N,M,4]
    rois = np.concatenate([b[:, None, :], ctx], axis=1)       # [N,9,4]
    wts = np.full((N, ROIS_PER_GROUP), np.float32(1.0 / M), dtype=np.float32)
    wts[:, 0] = np.float32(1.0)

    rois = rois[g0:g0 + GROUPS_PER_CORE].reshape(R_CORE, 4)
    wts = wts[g0:g0 + GROUPS_PER_CORE].reshape(R_CORE)
    x1, y1, x2, y2 = rois[:, 0], rois[:, 1], rois[:, 2], rois[:, 3]
    roi_w = np.maximum(x2 - x1, np.float32(1.0))
    roi_h = np.maximum(y2 - y1, np.float32(1.0))
    Ay = _axis_weights(y1, roi_h, H)                          # [R,P,H]
    Ax = _axis_weights(x1, roi_w, W) * wts[:, None, None]     # [R,P,W]

    AyT = np.ascontiguousarray(Ay.transpose(2, 0, 1).reshape(H, RP))
    # AxBD [128, R*16] block-diagonal per roi: rows w hold Ax[r,q,w] at
    # col r*16+q; rows 64+w hold the same at col r*16+8+q.
    AxBD = np.zeros((128, R_CORE * 16), dtype=np.float32)
    AxT = Ax.transpose(2, 0, 1)                               # [W, R, P]
    for psi in range(2):
        blk = AxBD[psi * 64:psi * 64 + W].reshape(W, R_CORE, 16)
        blk[:, :, psi * 8:psi * 8 + P] = AxT

    # K (contraction over h) is zero-padded 56->128: K=128 matmuls keep the
    # PE HAM activity monitor in the unthrottled 2.4 GHz state; K=56 ones
    # run at 1.2 GHz forever.
    F3 = np.zeros((128, 128, 128), dtype=np.float32)
    fmT = fm_b.transpose(1, 0, 2)                              # [h, c, w]
    F3[:H, :, 0:56] = fmT[:, 0::2, :]
    F3[:H, :, 64:120] = fmT[:, 1::2, :]
    AyT128 = np.zeros((128, RP), dtype=np.float32)
    AyT128[:H] = AyT
    return (F3.astype(BF16), AyT128.astype(BF16), AxBD.astype(BF16))


def _unpack_core_out(OUT):
    """OUT [2, 16, 8, 8, 112] -> [16, 256, 7, 7].

    OUT[kk, psi*8 + q, gb, win, il*7 + p] = out[g=2*gb+kk,
    c=2*(win*16+il)+psi, p, q].
    """
    a = OUT.reshape(2, 2, 8, 8, NWIN, WIN, P)[:, :, :P]   # [kk,psi,q,gb,win,il,p]
    a = a.transpose(3, 0, 4, 5, 1, 6, 2)                  # [gb,kk,win,il,psi,p,q]
    return np.ascontiguousarray(a.reshape(GROUPS_PER_CORE, C, P, P))


# ---------------------------------------------------------------- program

_PROGRAM = None


def _build_program():
    import concourse.bacc as bacc
    import concourse.tile as tile
    import concourse.mybir as mybir

    f32 = mybir.dt.float32
    bf16 = mybir.dt.bfloat16

    nc = bacc.Bacc("TRN2", target_bir_lowering=False, debug=False,
                   enable_asserts=False)
    f3_d = nc.dram_tensor("f3", [128, 128, 128], bf16, kind="ExternalInput").ap()
    ayt_d = nc.dram_tensor("ayt", [128, RP], bf16, kind="ExternalInput").ap()
    axbd_d = nc.dram_tensor("axbd", [128, R_CORE * 16], bf16,
                            kind="ExternalInput").ap()
    out_d = nc.dram_tensor("out", [2, 16, 8, NWIN, 112], f32,
                           kind="ExternalOutput").ap()

    with tile.TileContext(nc) as tc:
        with tc.tile_pool(name="const", bufs=1) as cpool, \
             tc.tile_pool(name="fmw", bufs=2) as fpool, \
             tc.tile_pool(name="tmp", bufs=2) as tpool, \
             tc.tile_pool(name="outp", bufs=1) as opool, \
             tc.tile_pool(name="ps1", bufs=2, space="PSUM") as ps1p, \
             tc.tile_pool(name="ps2", bufs=4, space="PSUM") as ps2p:

            AyT = cpool.tile([128, RP], bf16)
            for d in range(4):
                nc.sync.dma_start(AyT[:, 252 * d:252 * (d + 1)],
                                  ayt_d[:, 252 * d:252 * (d + 1)])
            AxBD = cpool.tile([128, R_CORE * 16], bf16)
            for d in range(4):
                nc.sync.dma_start(AxBD[:, 576 * d:576 * (d + 1)],
                                  axbd_d[:, 576 * d:576 * (d + 1)])
            OUT = opool.tile([128, 8, NWIN, 112], f32)

            def stage2_block(TMP, gb, win):
                # one pair of box groups: 18 accumulating matmuls + copy out
                ps2 = ps2p.tile([128, 112], f32, tag="ps2")
                for k in range(2):
                    g = gb * 2 + k
                    for j in range(ROIS_PER_GROUP):
                        r = g * ROIS_PER_GROUP + j
                        nc.tensor.matmul(
                            ps2[64 * k:64 * k + 16, :],
                            AxBD[:, r * 16:(r + 1) * 16],
                            TMP[:, r, :],
                            start=(j == 0), stop=(j == ROIS_PER_GROUP - 1))
                if gb % 2 == 0:
                    nc.vector.tensor_copy(out=OUT[:, gb, win, :], in_=ps2[:])
                else:
                    nc.scalar.copy(out=OUT[:, gb, win, :], in_=ps2[:])

            ncopy = 0
            prev = None   # (tmps, win) of the previous window
            for win in range(NWIN):
                F3w = fpool.tile([128, WIN, 128], bf16, tag="f3w")
                for d in range(8):
                    nc.sync.dma_start(
                        F3w[:, 2 * d:2 * (d + 1), :],
                        f3_d[:, win * WIN + 2 * d:win * WIN + 2 * (d + 1), :])
                TMP = tpool.tile([128, R_CORE, WIN * P], bf16, tag="tmp")
                # interleave one stage-2 block of the previous window after
                # every 2 stage-1 il iterations (4 matmuls): the narrow
                # stage-2 streams give the psum->TMP copies time to catch up.
                for il in range(WIN):
                    ps = ps1p.tile([128, 2, 512], f32, tag="ps1")
                    for ch in range(2):
                        nc.tensor.matmul(
                            ps[:, ch, 0:NCHUNK],
                            F3w[:, il, :],
                            AyT[:, ch * NCHUNK:(ch + 1) * NCHUNK],
                            start=True, stop=True)
                    dst = TMP[:, :, il * P:(il + 1) * P]
                    if ncopy % 2 == 0:
                        nc.vector.tensor_copy(out=dst, in_=ps[:, :, 0:NCHUNK])
                    else:
                        nc.scalar.copy(out=dst, in_=ps[:, :, 0:NCHUNK])
                    ncopy += 1
                    if prev is not None and il % 2 == 1:
                        stage2_block(prev[0], il // 2, prev[1])
                if prev is not None:
                    pwin = prev[1]
                    nc.sync.dma_start(out_d[0][:, :, pwin, :],
                                      OUT[0:16, :, pwin, :])
                    nc.sync.dma_start(out_d[1][:, :, pwin, :],
                                      OUT[64:80, :, pwin, :])
                prev = (TMP, win)
            for gb in range(8):
                stage2_block(prev[0], gb, prev[1])
            nc.sync.dma_start(out_d[0][:, :, NWIN - 1, :],
                              OUT[0:16, :, NWIN - 1, :])
            nc.sync.dma_start(out_d[1][:, :, NWIN - 1, :],
                              OUT[64:80, :, NWIN - 1, :])

    nc.compile()
    return nc


LAST_RESULT = None


def _ensure_axon_hooks_shim():
    """concourse's axon trace path imports antenv.axon_hooks, which this
    image's antenv package lacks; provide a minimal registry so a stray
    BASS_TRACE=1 in the environment cannot crash the kernel."""
    try:
        import antenv  # noqa: F401
        import antenv.axon_hooks  # noqa: F401
        return
    except ImportError:
        pass
    try:
        import sys
        import types
        import antenv
        mod = types.ModuleType("antenv.axon_hooks")
        mod._hook = None
        mod.get_axon_ntff_profile_hook = lambda: mod._hook

        def _set(h):
            mod._hook = h

        mod.set_axon_ntff_profile_hook = _set
        sys.modules["antenv.axon_hooks"] = mod
        antenv.axon_hooks = mod
    except Exception:
        pass


def kernel(feature_map, boxes, gt_boxes):
    global _PROGRAM, LAST_RESULT
    _ensure_axon_hooks_shim()
    feature_map = np.asarray(feature_map, dtype=np.float32)
    boxes = np.asarray(boxes, dtype=np.float32)
    gt_boxes = np.asarray(gt_boxes, dtype=np.float32)

    from concourse.bass_utils import run_bass_kernel_spmd

    if _PROGRAM is None:
        _PROGRAM = _build_program()
    nc = _PROGRAM

    in_maps = []
    for k in range(NCORES):
        b = k // 4
        g0 = (k % 4) * GROUPS_PER_CORE
        F3, AyT, AxBD = _prep_core(feature_map[b], boxes[b], gt_boxes[b], g0)
        in_maps.append({"f3": F3, "ayt": AyT, "axbd": AxBD})

    trace = bool(int(os.environ.get("ROI_TRACE", "0")))
    res = run_bass_kernel_spmd(nc, in_maps, list(range(NCORES)), trace=trace)
    LAST_RESULT = res

    out = np.zeros((B, N, C, P, P), dtype=np.float32)
    for k in range(NCORES):
        b = k // 4
        g0 = (k % 4) * GROUPS_PER_CORE
        out[b, g0:g0 + GROUPS_PER_CORE] = _unpack_core_out(res.results[k]["out"])
    return out


# revision 28
# speedup vs baseline: 1.1685x; 1.1685x over previous
"""ContextualRoIAlign Trainium2 kernel (v2: bf16 + streamed stage-2).

Problem (hardcoded): B=2, C=256, H=W=56, N=64 boxes, M=8 gt boxes, P=7.
out[b,n,c,p,q] = roi_align(fm[b], box_n)[c,p,q]
                 + mean_m roi_align(fm[b], union(box_n, gt_m))[c,p,q]

Decomposition: roi_align separates per axis into small interpolation
matrices Ay, Ax ([R,7,56], host-precomputed exactly like the reference):
  out[r,c,p,q] = sum_h Ay[r,p,h] * (sum_w fm[c,h,w] * Ax[r,q,w])
The 1/M mean weight is folded into Ax of the context rois, and the 9-roi
group sum is accumulated in PSUM.

Sharding: 8 cores; core k handles image k//4, box groups [16*(k%4), +16)
=> 144 rois per core (16 groups x (1 box + 8 ctx)). fm replicated per
image (4 cores each).

Device program per core (bf16 matmul inputs, fp32 psum accumulate):
  Stage 1 (contract h): weights = fm channel-pair [h=56, 128] where col
    c_loc*64+w holds fm[2i+c_loc, h, w]; rhs = AyT [56, 504] (72 rois'
    (r,p) columns per chunk); psum [128, 504] -> TMP[128, 72, 112] bf16.
    TMP partition c_loc*64+w holds tmp[c=2*(win*16+il)+c_loc, w] at free
    col (r_loc, il*7+p): w on partitions, no inter-stage transpose.
  Stage 2 (contract w): stationary = AxBD[:, r*16:+16] ([128,16] block-
    diagonal: rows 0:56 have Ax[r] in cols 0:8, rows 64:120 have Ax[r]
    in cols 8:16 => the 16-col LDWEIGHTS hides under the previous
    matmul); moving = TMP[:, r_loc, :] ([128, 112]); out [16,112].
    9 rois of a group accumulate in one psum chain; 4 groups share one
    psum tile at PE col-tile offsets 0/32/64/96.
"""
import os
import numpy as np
import ml_dtypes

P = 7
B, C, H, W, N, M = 2, 256, 56, 56, 64, 8
NCORES = 8
GROUPS_PER_CORE = 16
ROIS_PER_GROUP = 9
R_CORE = GROUPS_PER_CORE * ROIS_PER_GROUP   # 144
RP = R_CORE * P                              # 1008
WIN = 16                                     # channel pairs per window
NWIN = 128 // WIN                            # 8
NCHUNK = 504                                 # stage-1 rhs cols per matmul
RCHUNK = NCHUNK // P                         # 72 rois per TMP tile

BF16 = ml_dtypes.bfloat16


# ---------------------------------------------------------------- host prep

def _axis_weights(start, length, dim):
    """Exact numpy port of the reference's _axis_weights (float32)."""
    start = start.astype(np.float32)
    length = length.astype(np.float32)
    R = start.shape[0]
    S = int(np.ceil(dim / P))
    bin_sz = length / np.float32(P)
    grid = np.ceil(length / np.float32(P)).astype(np.int32)
    g = grid.astype(np.float32)[:, None, None]
    s = np.arange(S, dtype=np.float32)
    ph = np.arange(P, dtype=np.float32)
    coord = (start[:, None, None] + ph[None, :, None] * bin_sz[:, None, None]
             + (s[None, None, :] + np.float32(0.5)) * bin_sz[:, None, None] / g)
    valid = (coord >= -1.0) & (coord <= dim)
    c = np.maximum(coord, np.float32(0.0))
    low = np.floor(c).astype(np.int32)
    hi_clamp = low >= dim - 1
    low = np.where(hi_clamp, dim - 1, low)
    high = np.where(hi_clamp, dim - 1, low + 1)
    cv = np.where(hi_clamp, low.astype(np.float32), c)
    l = cv - low.astype(np.float32)
    smask = (s[None, None, :] < g) & valid
    w = smask.astype(np.float32) / g
    w_low = ((np.float32(1.0) - l) * w).astype(np.float32)
    w_high = (l * w).astype(np.float32)
    A = np.zeros((R, P, dim), dtype=np.float32)
    r_idx = np.broadcast_to(np.arange(R)[:, None, None], low.shape)
    p_idx = np.broadcast_to(np.arange(P)[None, :, None], low.shape)
    np.add.at(A, (r_idx, p_idx, low), w_low)
    np.add.at(A, (r_idx, p_idx, high), w_high)
    return A


def _prep_core(fm_b, boxes_b, gt_b, g0):
    b = boxes_b.astype(np.float32)
    g = gt_b.astype(np.float32)
    x1 = np.minimum(b[:, None, 0], g[None, :, 0])
    y1 = np.minimum(b[:, None, 1], g[None, :, 1])
    x2 = np.maximum(b[:, None, 2], g[None, :, 2])
    y2 = np.maximum(b[:, None, 3], g[None, :, 3])
    ctx = np.stack([x1, y1, x2, y2], axis=-1)                 # [N,M,4]
    rois = np.concatenate([b[:, None, :], ctx], axis=1)       # [N,9,4]
    wts = np.full((N, ROIS_PER_GROUP), np.float32(1.0 / M), dtype=np.float32)
    wts[:, 0] = np.float32(1.0)

    rois = rois[g0:g0 + GROUPS_PER_CORE].reshape(R_CORE, 4)
    wts = wts[g0:g0 + GROUPS_PER_CORE].reshape(R_CORE)
    x1, y1, x2, y2 = rois[:, 0], rois[:, 1], rois[:, 2], rois[:, 3]
    roi_w = np.maximum(x2 - x1, np.float32(1.0))
    roi_h = np.maximum(y2 - y1, np.float32(1.0))
    Ay = _axis_weights(y1, roi_h, H)                          # [R,P,H]
    Ax = _axis_weights(x1, roi_w, W) * wts[:, None, None]     # [R,P,W]

    AyT = np.ascontiguousarray(Ay.transpose(2, 0, 1).reshape(H, RP))
    # AxBD [128, R*16] block-diagonal per roi: rows w hold Ax[r,q,w] at
    # col r*16+q; rows 64+w hold the same at col r*16+8+q.
    AxBD = np.zeros((128, R_CORE * 16), dtype=np.float32)
    AxT = Ax.transpose(2, 0, 1)                               # [W, R, P]
    for psi in range(2):
        blk = AxBD[psi * 64:psi * 64 + W].reshape(W, R_CORE, 16)
        blk[:, :, psi * 8:psi * 8 + P] = AxT

    # K (contraction over h) is zero-padded 56->128: K=128 matmuls keep the
    # PE HAM activity monitor in the unthrottled 2.4 GHz state; K=56 ones
    # run at 1.2 GHz forever.
    F3 = np.zeros((128, 128, 128), dtype=np.float32)
    fmT = fm_b.transpose(1, 0, 2)                              # [h, c, w]
    F3[:H, :, 0:56] = fmT[:, 0::2, :]
    F3[:H, :, 64:120] = fmT[:, 1::2, :]
    AyT128 = np.zeros((128, RP), dtype=np.float32)
    AyT128[:H] = AyT
    return (F3.astype(BF16), AyT128.astype(BF16), AxBD.astype(BF16))


def _unpack_core_out(OUT):
    """OUT [2, 16, 8, 8, 112] -> [16, 256, 7, 7].

    OUT[kk, psi*8 + q, gb, win, il*7 + p] = out[g=2*gb+kk,
    c=2*(win*16+il)+psi, p, q].
    """
    a = OUT.reshape(2, 2, 8, 8, NWIN, WIN, P)[:, :, :P]   # [kk,psi,q,gb,win,il,p]
    a = a.transpose(3, 0, 4, 5, 1, 6, 2)                  # [gb,kk,win,il,psi,p,q]
    return np.ascontiguousarray(a.reshape(GROUPS_PER_CORE, C, P, P))


# ---------------------------------------------------------------- program

_PROGRAM = None


def _build_program():
    import concourse.bacc as bacc
    import concourse.tile as tile
    import concourse.mybir as mybir

    f32 = mybir.dt.float32
    bf16 = mybir.dt.bfloat16

    nc = bacc.Bacc("TRN2", target_bir_lowering=False, debug=False,
                   enable_asserts=False)
    f3_d = nc.dram_tensor("f3", [128, 128, 128], bf16, kind="ExternalInput").ap()
    ayt_d = nc.dram_tensor("ayt", [128, RP], bf16, kind="ExternalInput").ap()
    axbd_d = nc.dram_tensor("axbd", [128, R_CORE * 16], bf16,
                            kind="ExternalInput").ap()
    out_d = nc.dram_tensor("out", [2, 16, 8, NWIN, 112], f32,
                           kind="ExternalOutput").ap()

    with tile.TileContext(nc) as tc:
        with tc.tile_pool(name="const", bufs=1) as cpool, \
             tc.tile_pool(name="fmw", bufs=2) as fpool, \
             tc.tile_pool(name="tmp", bufs=2) as tpool, \
             tc.tile_pool(name="outp", bufs=1) as opool, \
             tc.tile_pool(name="ps1", bufs=5, space="PSUM") as ps1p, \
             tc.tile_pool(name="ps2", bufs=3, space="PSUM") as ps2p:

            AyT = cpool.tile([128, RP], bf16)
            for d in range(4):
                nc.sync.dma_start(AyT[:, 252 * d:252 * (d + 1)],
                                  ayt_d[:, 252 * d:252 * (d + 1)])
            AxBD = cpool.tile([128, R_CORE * 16], bf16)
            for d in range(4):
                nc.sync.dma_start(AxBD[:, 576 * d:576 * (d + 1)],
                                  axbd_d[:, 576 * d:576 * (d + 1)])
            OUT = opool.tile([128, 8, NWIN, 112], f32)

            def stage2_block(tmps, gb, win):
                # one pair of box groups: 18 accumulating matmuls + copy out
                ps2 = ps2p.tile([128, 112], f32, tag="ps2")
                for k in range(2):
                    g = gb * 2 + k
                    for j in range(ROIS_PER_GROUP):
                        r = g * ROIS_PER_GROUP + j
                        ch, rloc = divmod(r, RCHUNK)
                        nc.tensor.matmul(
                            ps2[64 * k:64 * k + 16, :],
                            AxBD[:, r * 16:(r + 1) * 16],
                            tmps[ch][:, rloc, :],
                            start=(j == 0), stop=(j == ROIS_PER_GROUP - 1))
                if gb % 2 == 0:
                    nc.vector.tensor_copy(out=OUT[:, gb, win, :], in_=ps2[:])
                else:
                    nc.scalar.copy(out=OUT[:, gb, win, :], in_=ps2[:])

            ncopy = 0
            prev = None   # (tmps, win) of the previous window
            for win in range(NWIN):
                F3w = fpool.tile([128, WIN, 128], bf16, tag="f3w")
                for d in range(8):
                    nc.sync.dma_start(
                        F3w[:, 2 * d:2 * (d + 1), :],
                        f3_d[:, win * WIN + 2 * d:win * WIN + 2 * (d + 1), :])
                TMP0 = tpool.tile([128, RCHUNK, WIN * P], bf16, tag="tmp0")
                TMP1 = tpool.tile([128, RCHUNK, WIN * P], bf16, tag="tmp1")
                tmps = [TMP0, TMP1]
                # interleave one stage-2 block of the previous window after
                # every 4 stage-1 matmuls: the narrow stage-2 streams give
                # the psum->TMP copies time to catch up with the matmuls.
                nmm = 0
                for ch in range(2):
                    TMP = tmps[ch]
                    for il in range(WIN):
                        ps = ps1p.tile([128, NCHUNK], f32, tag="ps1")
                        nc.tensor.matmul(
                            ps[:],
                            F3w[:, il, :],
                            AyT[:, ch * NCHUNK:(ch + 1) * NCHUNK],
                            start=True, stop=True)
                        dst = TMP[:, :, il * P:(il + 1) * P]
                        if ncopy % 2 == 0:
                            nc.vector.tensor_copy(out=dst, in_=ps[:])
                        else:
                            nc.scalar.copy(out=dst, in_=ps[:])
                        ncopy += 1
                        nmm += 1
                        if prev is not None and nmm % 4 == 0:
                            stage2_block(prev[0], nmm // 4 - 1, prev[1])
                if prev is not None:
                    pwin = prev[1]
                    nc.sync.dma_start(out_d[0][:, :, pwin, :],
                                      OUT[0:16, :, pwin, :])
                    nc.sync.dma_start(out_d[1][:, :, pwin, :],
                                      OUT[64:80, :, pwin, :])
                prev = (tmps, win)
            for gb in range(8):
                stage2_block(prev[0], gb, prev[1])
            nc.sync.dma_start(out_d[0][:, :, NWIN - 1, :],
                              OUT[0:16, :, NWIN - 1, :])
            nc.sync.dma_start(out_d[1][:, :, NWIN - 1, :],
                              OUT[64:80, :, NWIN - 1, :])

    nc.compile()
    return nc


LAST_RESULT = None


def _ensure_axon_hooks_shim():
    """concourse's axon trace path imports antenv.axon_hooks, which this
    image's antenv package lacks; provide a minimal registry so a stray
    BASS_TRACE=1 in the environment cannot crash the kernel."""
    try:
        import antenv  # noqa: F401
        import antenv.axon_hooks  # noqa: F401
        return
    except ImportError:
        pass
    try:
        import sys
        import types
        import antenv
        mod = types.ModuleType("antenv.axon_hooks")
        mod._hook = None
        mod.get_axon_ntff_profile_hook = lambda: mod._hook

        def _set(h):
            mod._hook = h

        mod.set_axon_ntff_profile_hook = _set
        sys.modules["antenv.axon_hooks"] = mod
        antenv.axon_hooks = mod
    except Exception:
        pass


def kernel(feature_map, boxes, gt_boxes):
    global _PROGRAM, LAST_RESULT
    _ensure_axon_hooks_shim()
    feature_map = np.asarray(feature_map, dtype=np.float32)
    boxes = np.asarray(boxes, dtype=np.float32)
    gt_boxes = np.asarray(gt_boxes, dtype=np.float32)

    from concourse.bass_utils import run_bass_kernel_spmd

    if _PROGRAM is None:
        _PROGRAM = _build_program()
    nc = _PROGRAM

    in_maps = []
    for k in range(NCORES):
        b = k // 4
        g0 = (k % 4) * GROUPS_PER_CORE
        F3, AyT, AxBD = _prep_core(feature_map[b], boxes[b], gt_boxes[b], g0)
        in_maps.append({"f3": F3, "ayt": AyT, "axbd": AxBD})

    trace = bool(int(os.environ.get("ROI_TRACE", "0")))
    res = run_bass_kernel_spmd(nc, in_maps, list(range(NCORES)), trace=trace)
    LAST_RESULT = res

    out = np.zeros((B, N, C, P, P), dtype=np.float32)
    for k in range(NCORES):
        b = k // 4
        g0 = (k % 4) * GROUPS_PER_CORE
        out[b, g0:g0 + GROUPS_PER_CORE] = _unpack_core_out(res.results[k]["out"])
    return out


# revision 30
# speedup vs baseline: 1.1983x; 1.0255x over previous
"""ContextualRoIAlign Trainium2 kernel (v2: bf16 + streamed stage-2).

Problem (hardcoded): B=2, C=256, H=W=56, N=64 boxes, M=8 gt boxes, P=7.
out[b,n,c,p,q] = roi_align(fm[b], box_n)[c,p,q]
                 + mean_m roi_align(fm[b], union(box_n, gt_m))[c,p,q]

Decomposition: roi_align separates per axis into small interpolation
matrices Ay, Ax ([R,7,56], host-precomputed exactly like the reference):
  out[r,c,p,q] = sum_h Ay[r,p,h] * (sum_w fm[c,h,w] * Ax[r,q,w])
The 1/M mean weight is folded into Ax of the context rois, and the 9-roi
group sum is accumulated in PSUM.

Sharding: 8 cores; core k handles image k//4, box groups [16*(k%4), +16)
=> 144 rois per core (16 groups x (1 box + 8 ctx)). fm replicated per
image (4 cores each).

Device program per core (bf16 matmul inputs, fp32 psum accumulate):
  Stage 1 (contract h): weights = fm channel-pair [h=56, 128] where col
    c_loc*64+w holds fm[2i+c_loc, h, w]; rhs = AyT [56, 504] (72 rois'
    (r,p) columns per chunk); psum [128, 504] -> TMP[128, 72, 112] bf16.
    TMP partition c_loc*64+w holds tmp[c=2*(win*16+il)+c_loc, w] at free
    col (r_loc, il*7+p): w on partitions, no inter-stage transpose.
  Stage 2 (contract w): stationary = AxBD[:, r*16:+16] ([128,16] block-
    diagonal: rows 0:56 have Ax[r] in cols 0:8, rows 64:120 have Ax[r]
    in cols 8:16 => the 16-col LDWEIGHTS hides under the previous
    matmul); moving = TMP[:, r_loc, :] ([128, 112]); out [16,112].
    9 rois of a group accumulate in one psum chain; 4 groups share one
    psum tile at PE col-tile offsets 0/32/64/96.
"""
import os
import numpy as np
import ml_dtypes

P = 7
B, C, H, W, N, M = 2, 256, 56, 56, 64, 8
NCORES = 8
GROUPS_PER_CORE = 16
ROIS_PER_GROUP = 9
R_CORE = GROUPS_PER_CORE * ROIS_PER_GROUP   # 144
RP = R_CORE * P                              # 1008
WIN = 16                                     # channel pairs per window
NWIN = 128 // WIN                            # 8
NCHUNK = 504                                 # stage-1 rhs cols per matmul
RCHUNK = NCHUNK // P                         # 72 rois per TMP tile

BF16 = ml_dtypes.bfloat16


# ---------------------------------------------------------------- host prep

def _axis_weights(start, length, dim):
    """Exact numpy port of the reference's _axis_weights (float32)."""
    start = start.astype(np.float32)
    length = length.astype(np.float32)
    R = start.shape[0]
    S = int(np.ceil(dim / P))
    bin_sz = length / np.float32(P)
    grid = np.ceil(length / np.float32(P)).astype(np.int32)
    g = grid.astype(np.float32)[:, None, None]
    s = np.arange(S, dtype=np.float32)
    ph = np.arange(P, dtype=np.float32)
    coord = (start[:, None, None] + ph[None, :, None] * bin_sz[:, None, None]
             + (s[None, None, :] + np.float32(0.5)) * bin_sz[:, None, None] / g)
    valid = (coord >= -1.0) & (coord <= dim)
    c = np.maximum(coord, np.float32(0.0))
    low = np.floor(c).astype(np.int32)
    hi_clamp = low >= dim - 1
    low = np.where(hi_clamp, dim - 1, low)
    high = np.where(hi_clamp, dim - 1, low + 1)
    cv = np.where(hi_clamp, low.astype(np.float32), c)
    l = cv - low.astype(np.float32)
    smask = (s[None, None, :] < g) & valid
    w = smask.astype(np.float32) / g
    w_low = ((np.float32(1.0) - l) * w).astype(np.float32)
    w_high = (l * w).astype(np.float32)
    A = np.zeros((R, P, dim), dtype=np.float32)
    r_idx = np.broadcast_to(np.arange(R)[:, None, None], low.shape)
    p_idx = np.broadcast_to(np.arange(P)[None, :, None], low.shape)
    np.add.at(A, (r_idx, p_idx, low), w_low)
    np.add.at(A, (r_idx, p_idx, high), w_high)
    return A


def _prep_core(fm_b, boxes_b, gt_b, g0):
    b = boxes_b.astype(np.float32)
    g = gt_b.astype(np.float32)
    x1 = np.minimum(b[:, None, 0], g[None, :, 0])
    y1 = np.minimum(b[:, None, 1], g[None, :, 1])
    x2 = np.maximum(b[:, None, 2], g[None, :, 2])
    y2 = np.maximum(b[:, None, 3], g[None, :, 3])
    ctx = np.stack([x1, y1, x2, y2], axis=-1)                 # [N,M,4]
    rois = np.concatenate([b[:, None, :], ctx], axis=1)       # [N,9,4]
    wts = np.full((N, ROIS_PER_GROUP), np.float32(1.0 / M), dtype=np.float32)
    wts[:, 0] = np.float32(1.0)

    rois = rois[g0:g0 + GROUPS_PER_CORE].reshape(R_CORE, 4)
    wts = wts[g0:g0 + GROUPS_PER_CORE].reshape(R_CORE)
    x1, y1, x2, y2 = rois[:, 0], rois[:, 1], rois[:, 2], rois[:, 3]
    roi_w = np.maximum(x2 - x1, np.float32(1.0))
    roi_h = np.maximum(y2 - y1, np.float32(1.0))
    Ay = _axis_weights(y1, roi_h, H)                          # [R,P,H]
    Ax = _axis_weights(x1, roi_w, W) * wts[:, None, None]     # [R,P,W]

    AyT = np.ascontiguousarray(Ay.transpose(2, 0, 1).reshape(H, RP))
    # AxBD [128, R*16] block-diagonal per roi: rows w hold Ax[r,q,w] at
    # col r*16+q; rows 64+w hold the same at col r*16+8+q.
    AxBD = np.zeros((128, R_CORE * 16), dtype=np.float32)
    AxT = Ax.transpose(2, 0, 1)                               # [W, R, P]
    for psi in range(2):
        blk = AxBD[psi * 64:psi * 64 + W].reshape(W, R_CORE, 16)
        blk[:, :, psi * 8:psi * 8 + P] = AxT

    # K (contraction over h) is zero-padded 56->128: K=128 matmuls keep the
    # PE HAM activity monitor in the unthrottled 2.4 GHz state; K=56 ones
    # run at 1.2 GHz forever.
    F3 = np.zeros((128, 128, 128), dtype=np.float32)
    fmT = fm_b.transpose(1, 0, 2)                              # [h, c, w]
    F3[:H, :, 0:56] = fmT[:, 0::2, :]
    F3[:H, :, 64:120] = fmT[:, 1::2, :]
    AyT128 = np.zeros((128, RP), dtype=np.float32)
    AyT128[:H] = AyT
    return (F3.astype(BF16), AyT128.astype(BF16), AxBD.astype(BF16))


def _unpack_core_out(OUT):
    """OUT [2, 16, 8, 8, 112] -> [16, 256, 7, 7].

    OUT[kk, psi*8 + q, gb, win, il*7 + p] = out[g=2*gb+kk,
    c=2*(win*16+il)+psi, p, q].
    """
    a = OUT.reshape(2, 2, 8, 8, NWIN, WIN, P)[:, :, :P]   # [kk,psi,q,gb,win,il,p]
    a = a.transpose(3, 0, 4, 5, 1, 6, 2)                  # [gb,kk,win,il,psi,p,q]
    return np.ascontiguousarray(a.reshape(GROUPS_PER_CORE, C, P, P))


# ---------------------------------------------------------------- program

_PROGRAM = None


def _build_program():
    import concourse.bacc as bacc
    import concourse.tile as tile
    import concourse.mybir as mybir

    f32 = mybir.dt.float32
    bf16 = mybir.dt.bfloat16

    nc = bacc.Bacc("TRN2", target_bir_lowering=False, debug=False,
                   enable_asserts=False)
    f3_d = nc.dram_tensor("f3", [128, 128, 128], bf16, kind="ExternalInput").ap()
    ayt_d = nc.dram_tensor("ayt", [128, RP], bf16, kind="ExternalInput").ap()
    axbd_d = nc.dram_tensor("axbd", [128, R_CORE * 16], bf16,
                            kind="ExternalInput").ap()
    out_d = nc.dram_tensor("out", [2, 16, 8, NWIN, 112], f32,
                           kind="ExternalOutput").ap()

    with tile.TileContext(nc) as tc:
        with tc.tile_pool(name="const", bufs=1) as cpool, \
             tc.tile_pool(name="fmw", bufs=2) as fpool, \
             tc.tile_pool(name="tmp", bufs=2) as tpool, \
             tc.tile_pool(name="outp", bufs=1) as opool, \
             tc.tile_pool(name="ps1", bufs=5, space="PSUM") as ps1p, \
             tc.tile_pool(name="ps2", bufs=3, space="PSUM") as ps2p:

            AyT = cpool.tile([128, RP], bf16)
            for d in range(4):
                nc.sync.dma_start(AyT[:, 252 * d:252 * (d + 1)],
                                  ayt_d[:, 252 * d:252 * (d + 1)])
            AxBD = cpool.tile([128, R_CORE * 16], bf16)
            OUT = opool.tile([128, 8, NWIN, 112], f32)

            def stage2_block(tmps, gb, win):
                # one pair of box groups: 18 accumulating matmuls + copy out
                ps2 = ps2p.tile([128, 112], f32, tag="ps2")
                for k in range(2):
                    g = gb * 2 + k
                    for j in range(ROIS_PER_GROUP):
                        r = g * ROIS_PER_GROUP + j
                        ch, rloc = divmod(r, RCHUNK)
                        nc.tensor.matmul(
                            ps2[64 * k:64 * k + 16, :],
                            AxBD[:, r * 16:(r + 1) * 16],
                            tmps[ch][:, rloc, :],
                            start=(j == 0), stop=(j == ROIS_PER_GROUP - 1))
                if gb % 2 == 0:
                    nc.vector.tensor_copy(out=OUT[:, gb, win, :], in_=ps2[:])
                else:
                    nc.scalar.copy(out=OUT[:, gb, win, :], in_=ps2[:])

            ncopy = 0
            prev = None   # (tmps, win) of the previous window
            for win in range(NWIN):
                F3w = fpool.tile([128, WIN, 128], bf16, tag="f3w")
                for d in range(8):
                    nc.sync.dma_start(
                        F3w[:, 2 * d:2 * (d + 1), :],
                        f3_d[:, win * WIN + 2 * d:win * WIN + 2 * (d + 1), :])
                if win == 0:
                    # AxBD is first needed by the stage-2 blocks of window 1;
                    # issuing its DMA after F3w(0) keeps the head short.
                    for d in range(4):
                        nc.sync.dma_start(AxBD[:, 576 * d:576 * (d + 1)],
                                          axbd_d[:, 576 * d:576 * (d + 1)])
                TMP0 = tpool.tile([128, RCHUNK, WIN * P], bf16, tag="tmp0")
                TMP1 = tpool.tile([128, RCHUNK, WIN * P], bf16, tag="tmp1")
                tmps = [TMP0, TMP1]
                # interleave one stage-2 block of the previous window after
                # every 4 stage-1 matmuls: the narrow stage-2 streams give
                # the psum->TMP copies time to catch up with the matmuls.
                nmm = 0
                for ch in range(2):
                    TMP = tmps[ch]
                    for il in range(WIN):
                        ps = ps1p.tile([128, NCHUNK], f32, tag="ps1")
                        nc.tensor.matmul(
                            ps[:],
                            F3w[:, il, :],
                            AyT[:, ch * NCHUNK:(ch + 1) * NCHUNK],
                            start=True, stop=True)
                        dst = TMP[:, :, il * P:(il + 1) * P]
                        if ncopy % 2 == 0:
                            nc.vector.tensor_copy(out=dst, in_=ps[:])
                        else:
                            nc.scalar.copy(out=dst, in_=ps[:])
                        ncopy += 1
                        nmm += 1
                        if prev is not None and nmm % 4 == 0:
                            stage2_block(prev[0], nmm // 4 - 1, prev[1])
                if prev is not None:
                    pwin = prev[1]
                    nc.sync.dma_start(out_d[0][:, :, pwin, :],
                                      OUT[0:16, :, pwin, :])
                    nc.sync.dma_start(out_d[1][:, :, pwin, :],
                                      OUT[64:80, :, pwin, :])
                prev = (tmps, win)
            for gb in range(8):
                stage2_block(prev[0], gb, prev[1])
            nc.sync.dma_start(out_d[0][:, :, NWIN - 1, :],
                              OUT[0:16, :, NWIN - 1, :])
            nc.sync.dma_start(out_d[1][:, :, NWIN - 1, :],
                              OUT[64:80, :, NWIN - 1, :])

    nc.compile()
    return nc


LAST_RESULT = None


def _ensure_axon_hooks_shim():
    """concourse's axon trace path imports antenv.axon_hooks, which this
    image's antenv package lacks; provide a minimal registry so a stray
    BASS_TRACE=1 in the environment cannot crash the kernel."""
    try:
        import antenv  # noqa: F401
        import antenv.axon_hooks  # noqa: F401
        return
    except ImportError:
        pass
    try:
        import sys
        import types
        import antenv
        mod = types.ModuleType("antenv.axon_hooks")
        mod._hook = None
        mod.get_axon_ntff_profile_hook = lambda: mod._hook

        def _set(h):
            mod._hook = h

        mod.set_axon_ntff_profile_hook = _set
        sys.modules["antenv.axon_hooks"] = mod
        antenv.axon_hooks = mod
    except Exception:
        pass


def kernel(feature_map, boxes, gt_boxes):
    global _PROGRAM, LAST_RESULT
    _ensure_axon_hooks_shim()
    feature_map = np.asarray(feature_map, dtype=np.float32)
    boxes = np.asarray(boxes, dtype=np.float32)
    gt_boxes = np.asarray(gt_boxes, dtype=np.float32)

    from concourse.bass_utils import run_bass_kernel_spmd

    if _PROGRAM is None:
        _PROGRAM = _build_program()
    nc = _PROGRAM

    in_maps = []
    for k in range(NCORES):
        b = k // 4
        g0 = (k % 4) * GROUPS_PER_CORE
        F3, AyT, AxBD = _prep_core(feature_map[b], boxes[b], gt_boxes[b], g0)
        in_maps.append({"f3": F3, "ayt": AyT, "axbd": AxBD})

    trace = bool(int(os.environ.get("ROI_TRACE", "0")))
    res = run_bass_kernel_spmd(nc, in_maps, list(range(NCORES)), trace=trace)
    LAST_RESULT = res

    out = np.zeros((B, N, C, P, P), dtype=np.float32)
    for k in range(NCORES):
        b = k // 4
        g0 = (k % 4) * GROUPS_PER_CORE
        out[b, g0:g0 + GROUPS_PER_CORE] = _unpack_core_out(res.results[k]["out"])
    return out
